# revision 1
# baseline (speedup 1.0000x reference)
"""BerHu loss kernel for Trainium2, 8-core data-parallel.

Reference computation (per sample n over its S = 1*480*640 elements):
    d  = pred - tgt
    c  = max|d| / 5
    berhu = |d|                 where |d| <= c
          = (d^2 + c^2) / (2c)  otherwise
    loss = mean_n mean_i berhu

Identity used on device:  berhu = |d| + relu(|d| - c)^2 * (1/(2c))
Two custom DVE ops do all heavy lifting (one pass each over the data):
  ABSDIFF:  ad = |p - t|            accum: mx = max(ad)      [per partition]
  BERHU:    junk = ad + relu(ad-c)^2 * i2c   accum: bh = sum [per partition]
The host sums the per-partition/per-sample bh partials:
    loss = sum(bh) / (N * S).

Sharding: pure data parallel, 8 samples per core on 8 cores; each
sample's 307200 elements are laid out [128 partitions x 2400].
"""

import numpy as np

N = 64          # batch
S = 307200      # 1*480*640 elements per sample
NCORES = 8
NLOC = N // NCORES   # samples per core
P = 128              # SBUF partitions
F = S // P           # 2400 columns per sample

_PROG = None


def _register_ops():
    import concourse.dve_ops as dve_ops
    from concourse.dve_ops import OPS, DveOp, has_src1
    from concourse.dve_spec import (C0, C1, C2, AluOp, Spec, Src0, Src1,
                                    Zero, lower)
    from concourse.dve_spec import relu, sq, maxx
    from concourse.dve_uop import DveOpSpec

    def add_op(name, spec):
        for o in OPS:
            if o.name == name:
                return o
        op = DveOp(name, spec, subdim=False, uops_sha={})
        OPS.append(op)
        dve_ops.CUSTOM_DVE_SPECS[name] = spec
        dve_ops._SUB_OPCODE_FOR_NAME[name] = (
            dve_ops._CUSTOM_DVE_ROW_BASE + len(OPS) - 1)
        assert dve_ops._SUB_OPCODE_FOR_NAME[name] < 0x20
        for ver in ("v3", "v4"):
            sha = DveOpSpec(
                name=name,
                opcode=dve_ops.get_dve_sub_opcode(name),
                uops=lower(spec, ver=ver),
                rd1_en=has_src1(spec),
            ).sha(ver)
            op.uops_sha[ver] = sha
        return op

    def _absdiff_ref(in0, in1, c0, c1, c2):
        x = in0.astype(np.float32).reshape(in0.shape[0], -1)
        y = np.asarray(in1, np.float32).reshape(in0.shape[0], -1)
        out = np.abs(x - y).astype(np.float32)
        return out, out.max(axis=-1)

    def _berhu_ref(in0, in1, c0, c1, c2):
        # c0 = c = m/5, c1 = 1/(2c) = 2.5/m
        x = in0.astype(np.float32).reshape(in0.shape[0], -1)
        r = np.maximum(x - c0, 0.0).astype(np.float32)
        out = (x + r * r * c1).astype(np.float32)
        return out, out.sum(axis=-1, dtype=np.float32)

    d = Src0 - Src1
    absdiff = add_op(
        "ANT_BERHU_ABSDIFF",
        Spec(body=maxx(d, Zero - d), accum=AluOp.MAX, reference=_absdiff_ref),
    )
    berhu = add_op(
        "ANT_BERHU_ACC",
        Spec(body=Src0 + sq(relu(Src0 - C0)) * C1, accum=AluOp.ADD,
             reference=_berhu_ref),
    )
    return absdiff, berhu


def _build(repeat=1, loop_n=None):
    """Build the per-core program. `repeat` > 1 replays the whole 8-sample
    body that many times inside one NEFF (unrolled); `loop_n` instead wraps
    the body in a device-side For_i loop (benchmarking only)."""
    from contextlib import ExitStack

    import concourse.bacc as bacc
    import concourse.tile as tile
    from concourse import mybir

    absdiff_op, berhu_op = _register_ops()

    f32 = mybir.dt.float32
    Alu = mybir.AluOpType

    nc = bacc.Bacc("TRN2", target_bir_lowering=False, debug=False,
                   num_devices=NCORES)
    p_d = nc.dram_tensor("p", [NLOC * P, F], f32, kind="ExternalInput").ap()
    t_d = nc.dram_tensor("t", [NLOC * P, F], f32, kind="ExternalInput").ap()
    bh_d = nc.dram_tensor("bh", [P, NLOC], f32, kind="ExternalOutput").ap()

    with tile.TileContext(nc) as tc, ExitStack() as ctx:
        io = ctx.enter_context(tc.tile_pool(name="io", bufs=3))
        work = ctx.enter_context(tc.tile_pool(name="work", bufs=3))
        work2 = ctx.enter_context(tc.tile_pool(name="work2", bufs=2))
        small = ctx.enter_context(tc.tile_pool(name="small", bufs=3))
        stats = ctx.enter_context(tc.tile_pool(name="stats", bufs=1))
        psum = ctx.enter_context(tc.tile_pool(name="psum", bufs=3,
                                              space="PSUM"))

        bh_t = stats.tile([P, NLOC], f32, tag="bh")
        ones_t = stats.tile([1, P], f32, tag="ones")
        nc.vector.memset(ones_t[:], 1.0)
        # identity matrix for PE cross-partition transpose
        ident = stats.tile([P, P], f32, tag="ident")
        nc.vector.memset(ident[:], 1.0)
        nc.gpsimd.affine_select(
            out=ident[:], in_=ident[:], pattern=[[-1, P]],
            compare_op=mybir.AluOpType.is_equal, fill=0.0,
            base=0, channel_multiplier=1,
        )
        total = NLOC * repeat

        pair = {}

        def load_pair(i):
            # one 2.4 MB DMA covers two consecutive samples (halves the
            # per-sample DMA instruction + completion-sem overhead)
            n = i % NLOC
            rows = slice(n * P, (n + 2) * P)
            pt = io.tile([P, 2 * F], f32, tag="p")
            tt = io.tile([P, 2 * F], f32, tag="t")
            src_p = p_d[rows, :].rearrange("(s p) f -> p s f", s=2)
            src_t = t_d[rows, :].rearrange("(s p) f -> p s f", s=2)
            nc.sync.dma_start(
                out=pt[:].rearrange("p (s f) -> p s f", s=2), in_=src_p)
            nc.scalar.dma_start(
                out=tt[:].rearrange("p (s f) -> p s f", s=2), in_=src_t)
            pair["p"], pair["t"] = pt, tt

        def pass1(i):
            if i % 2 == 0:
                load_pair(i)
            k = i % 2
            cols = slice(k * F, (k + 1) * F)
            # ad = |p - t|; mxn = per-partition max(ad)
            ad = work.tile([P, F], f32, tag="ad")
            mxn = small.tile([P, 1], f32, tag="mxn")
            nc.vector._custom_dve(absdiff_op, out=ad[:],
                                  in0=pair["p"][:, cols],
                                  in1=pair["t"][:, cols],
                                  accum_out=mxn[:])
            return {"ad": ad, "mxn": mxn}

        def chain(st):
            # cross-partition max: PE transpose (mxn^T @ I) -> [1, P] PSUM
            # row, DVE max-reduce -> scalar m; cpair = (m/5, 2.5/m); a K=1
            # ones-matmul broadcasts cpair to all 128 partitions in PSUM.
            mrow = psum.tile([1, P], f32, tag="mrow")
            nc.tensor.matmul(mrow[:], st["mxn"][:], ident[:],
                             start=True, stop=True)
            cpair = small.tile([1, 2], f32, tag="cpair")
            mr = small.tile([1, 3], f32, tag="mr")
            nc.vector.tensor_reduce(out=mr[0:1, 0:1], in_=mrow[:],
                                    axis=mybir.AxisListType.X, op=Alu.max)
            # floor m to avoid 1/0 when pred == tgt exactly (then bh = 0
            # correctly, since relu(0 - c) = 0)
            nc.vector.tensor_scalar_max(out=mr[0:1, 1:2],
                                        in0=mr[0:1, 0:1], scalar1=1e-20)
            nc.vector.reciprocal(out=mr[0:1, 2:3], in_=mr[0:1, 1:2])
            nc.vector.tensor_scalar_mul(out=cpair[0:1, 0:1],
                                        in0=mr[0:1, 1:2], scalar1=0.2)
            nc.vector.tensor_scalar_mul(out=cpair[0:1, 1:2],
                                        in0=mr[0:1, 2:3], scalar1=2.5)
            cb = psum.tile([P, 2], f32, tag="cb")
            nc.tensor.matmul(cb[:], ones_t[:], cpair[:],
                             start=True, stop=True)
            st["cb"] = cb

        def pass2(i, st):
            # bh[:, n] = sum(ad + relu(ad - c)^2 * i2c)
            n = i % NLOC
            junk = work2.tile([P, F], f32, tag="junk")
            nc.vector._custom_dve(berhu_op, out=junk[:], in0=st["ad"][:],
                                  s0=st["cb"][:, 0:1], s1=st["cb"][:, 1:2],
                                  accum_out=bh_t[:, n:n + 1])

        # 2-deep software pipeline: pass1(i) | chain(i-1) | pass2(i-2) keeps
        # the DVE stream free of waits on the c-derivation chain.
        def body():
            hist = {}
            for i in range(total):
                hist[i] = pass1(i)
                if i - 1 >= 0:
                    chain(hist[i - 1])
                if i - 2 >= 0:
                    pass2(i - 2, hist.pop(i - 2))
            for i in (total - 2, total - 1):
                if i >= 0:
                    if "cb" not in hist[i]:
                        chain(hist[i])
                    pass2(i, hist.pop(i))

        if loop_n is not None:
            with tc.For_i(0, loop_n, 1):
                body()
        else:
            body()

        nc.sync.dma_start(out=bh_d[:], in_=bh_t[:])

    nc.compile()
    return nc


def _get_prog():
    global _PROG
    if _PROG is None:
        _PROG = _build()
    return _PROG


def _combine(results):
    total = 0.0
    for r in results:
        total += r["bh"].astype(np.float64).sum()
    return np.float32(total / (N * S))


def kernel(predictions, targets):
    from concourse.bass_utils import run_bass_kernel_spmd

    nc = _get_prog()
    p = np.ascontiguousarray(
        np.asarray(predictions, dtype=np.float32).reshape(NCORES, NLOC * P, F))
    t = np.ascontiguousarray(
        np.asarray(targets, dtype=np.float32).reshape(NCORES, NLOC * P, F))
    in_maps = [{"p": p[k], "t": t[k]} for k in range(NCORES)]
    res = run_bass_kernel_spmd(nc, in_maps, list(range(NCORES)))
    return _combine(res.results)



# revision 3
# speedup vs baseline: 1.1906x; 1.1906x over previous
"""BerHu loss kernel for Trainium2, 8-core data-parallel.

Reference computation (per sample n over its S = 1*480*640 elements):
    d  = pred - tgt
    c  = max|d| / 5
    berhu = |d|                 where |d| <= c
          = (d^2 + c^2) / (2c)  otherwise
    loss = mean_n mean_i berhu

Identity used on device:  berhu = |d| + relu(|d| - c)^2 * (1/(2c))
Two custom DVE ops do all heavy lifting (one pass each over the data):
  ABSDIFF:  ad = |p - t|            accum: mx = max(ad)      [per partition]
  BERHU:    junk = ad + relu(ad-c)^2 * i2c   accum: bh = sum [per partition]
The host sums the per-partition/per-sample bh partials:
    loss = sum(bh) / (N * S).

Sharding: pure data parallel, 8 samples per core on 8 cores; each
sample's 307200 elements are laid out [128 partitions x 2400].
"""

import numpy as np

N = 64          # batch
S = 307200      # 1*480*640 elements per sample
NCORES = 8
NLOC = N // NCORES   # samples per core
P = 128              # SBUF partitions
F = S // P           # 2400 columns per sample

_PROG = None


def _register_ops():
    import concourse.dve_ops as dve_ops
    from concourse.dve_ops import OPS, DveOp, has_src1
    from concourse.dve_spec import (C0, C1, C2, AluOp, Spec, Src0, Src1,
                                    Zero, lower)
    from concourse.dve_spec import relu, sq, maxx
    from concourse.dve_uop import DveOpSpec

    def add_op(name, spec):
        for o in OPS:
            if o.name == name:
                return o
        op = DveOp(name, spec, subdim=False, uops_sha={})
        OPS.append(op)
        dve_ops.CUSTOM_DVE_SPECS[name] = spec
        dve_ops._SUB_OPCODE_FOR_NAME[name] = (
            dve_ops._CUSTOM_DVE_ROW_BASE + len(OPS) - 1)
        assert dve_ops._SUB_OPCODE_FOR_NAME[name] < 0x20
        for ver in ("v3", "v4"):
            sha = DveOpSpec(
                name=name,
                opcode=dve_ops.get_dve_sub_opcode(name),
                uops=lower(spec, ver=ver),
                rd1_en=has_src1(spec),
            ).sha(ver)
            op.uops_sha[ver] = sha
        return op

    def _absdiff_ref(in0, in1, c0, c1, c2):
        x = in0.astype(np.float32).reshape(in0.shape[0], -1)
        y = np.asarray(in1, np.float32).reshape(in0.shape[0], -1)
        out = np.abs(x - y).astype(np.float32)
        return out, out.max(axis=-1)

    def _berhu_ref(in0, in1, c0, c1, c2):
        # c0 = c = m/5, c1 = 1/(2c) = 2.5/m
        x = in0.astype(np.float32).reshape(in0.shape[0], -1)
        r = np.maximum(x - c0, 0.0).astype(np.float32)
        out = (x + r * r * c1).astype(np.float32)
        return out, out.sum(axis=-1, dtype=np.float32)

    d = Src0 - Src1
    absdiff = add_op(
        "ANT_BERHU_ABSDIFF",
        Spec(body=maxx(d, Zero - d), accum=AluOp.MAX, reference=_absdiff_ref),
    )
    berhu = add_op(
        "ANT_BERHU_ACC",
        Spec(body=Src0 + sq(relu(Src0 - C0)) * C1, accum=AluOp.ADD,
             reference=_berhu_ref),
    )
    return absdiff, berhu


def _build(repeat=1, loop_n=None, queues=("sync", "scalar")):
    """Build the per-core program. `repeat` > 1 replays the whole 8-sample
    body that many times inside one NEFF (unrolled); `loop_n` instead wraps
    the body in a device-side For_i loop (benchmarking only). `queues`:
    engine queues the paired input DMAs rotate over."""
    from contextlib import ExitStack

    import concourse.bacc as bacc
    import concourse.tile as tile
    from concourse import mybir

    absdiff_op, berhu_op = _register_ops()

    f32 = mybir.dt.float32
    Alu = mybir.AluOpType

    nc = bacc.Bacc("TRN2", target_bir_lowering=False, debug=False,
                   num_devices=NCORES)
    p_d = nc.dram_tensor("p", [NLOC * P, F], f32, kind="ExternalInput").ap()
    t_d = nc.dram_tensor("t", [NLOC * P, F], f32, kind="ExternalInput").ap()
    bh_d = nc.dram_tensor("bh", [P, NLOC], f32, kind="ExternalOutput").ap()

    with tile.TileContext(nc) as tc, ExitStack() as ctx:
        io = ctx.enter_context(tc.tile_pool(name="io", bufs=3))
        work = ctx.enter_context(tc.tile_pool(name="work", bufs=3))
        work2 = ctx.enter_context(tc.tile_pool(name="work2", bufs=2))
        small = ctx.enter_context(tc.tile_pool(name="small", bufs=3))
        stats = ctx.enter_context(tc.tile_pool(name="stats", bufs=1))
        psum = ctx.enter_context(tc.tile_pool(name="psum", bufs=3,
                                              space="PSUM"))

        bh_t = stats.tile([P, NLOC], f32, tag="bh")
        ones_t = stats.tile([1, P], f32, tag="ones")
        nc.vector.memset(ones_t[:], 1.0)
        # identity matrix for PE cross-partition transpose
        ident = stats.tile([P, P], f32, tag="ident")
        nc.vector.memset(ident[:], 1.0)
        nc.gpsimd.affine_select(
            out=ident[:], in_=ident[:], pattern=[[-1, P]],
            compare_op=mybir.AluOpType.is_equal, fill=0.0,
            base=0, channel_multiplier=1,
        )
        total = NLOC * repeat
        q_engines = [getattr(nc, q) for q in queues]

        pair = {}

        def load_pair(i):
            # one 2.4 MB DMA covers two consecutive samples (halves the
            # per-sample DMA instruction + completion-sem overhead)
            n = i % NLOC
            rows = slice(n * P, (n + 2) * P)
            pt = io.tile([P, 2 * F], f32, tag="p")
            tt = io.tile([P, 2 * F], f32, tag="t")
            src_p = p_d[rows, :].rearrange("(s p) f -> p s f", s=2)
            src_t = t_d[rows, :].rearrange("(s p) f -> p s f", s=2)
            q_engines[i % len(q_engines)].dma_start(
                out=pt[:].rearrange("p (s f) -> p s f", s=2), in_=src_p)
            q_engines[(i + 1) % len(q_engines)].dma_start(
                out=tt[:].rearrange("p (s f) -> p s f", s=2), in_=src_t)
            pair["p"], pair["t"] = pt, tt

        def pass1(i):
            if i % 2 == 0:
                load_pair(i)
            k = i % 2
            cols = slice(k * F, (k + 1) * F)
            # ad = |p - t|; mxn = per-partition max(ad)
            ad = work.tile([P, F], f32, tag="ad")
            mxn = small.tile([P, 1], f32, tag="mxn")
            nc.vector._custom_dve(absdiff_op, out=ad[:],
                                  in0=pair["p"][:, cols],
                                  in1=pair["t"][:, cols],
                                  accum_out=mxn[:])
            return {"ad": ad, "mxn": mxn}

        def chain(st):
            # cross-partition max: PE transpose (mxn^T @ I) -> [1, P] PSUM
            # row, DVE max-reduce -> scalar m; cpair = (m/5, 2.5/m); a K=1
            # ones-matmul broadcasts cpair to all 128 partitions in PSUM.
            mrow = psum.tile([1, P], f32, tag="mrow")
            nc.tensor.matmul(mrow[:], st["mxn"][:], ident[:],
                             start=True, stop=True)
            cpair = small.tile([1, 2], f32, tag="cpair")
            mr = small.tile([1, 3], f32, tag="mr")
            nc.vector.tensor_reduce(out=mr[0:1, 0:1], in_=mrow[:],
                                    axis=mybir.AxisListType.X, op=Alu.max)
            # floor m to avoid 1/0 when pred == tgt exactly (then bh = 0
            # correctly, since relu(0 - c) = 0)
            nc.vector.tensor_scalar_max(out=mr[0:1, 1:2],
                                        in0=mr[0:1, 0:1], scalar1=1e-20)
            nc.vector.reciprocal(out=mr[0:1, 2:3], in_=mr[0:1, 1:2])
            nc.vector.tensor_scalar_mul(out=cpair[0:1, 0:1],
                                        in0=mr[0:1, 1:2], scalar1=0.2)
            nc.vector.tensor_scalar_mul(out=cpair[0:1, 1:2],
                                        in0=mr[0:1, 2:3], scalar1=2.5)
            cb = psum.tile([P, 2], f32, tag="cb")
            nc.tensor.matmul(cb[:], ones_t[:], cpair[:],
                             start=True, stop=True)
            st["cb"] = cb

        def pass2(i, st):
            # bh[:, n] = sum(ad + relu(ad - c)^2 * i2c)
            n = i % NLOC
            junk = work2.tile([P, F], f32, tag="junk")
            nc.vector._custom_dve(berhu_op, out=junk[:], in0=st["ad"][:],
                                  s0=st["cb"][:, 0:1], s1=st["cb"][:, 1:2],
                                  accum_out=bh_t[:, n:n + 1])

        # 2-deep software pipeline: pass1(i) | chain(i-1) | pass2(i-2) keeps
        # the DVE stream free of waits on the c-derivation chain.
        def body():
            hist = {}
            for i in range(total):
                hist[i] = pass1(i)
                if i - 1 >= 0:
                    chain(hist[i - 1])
                if i - 2 >= 0:
                    pass2(i - 2, hist.pop(i - 2))
            for i in (total - 2, total - 1):
                if i >= 0:
                    if "cb" not in hist[i]:
                        chain(hist[i])
                    pass2(i, hist.pop(i))

        if loop_n is not None:
            with tc.For_i(0, loop_n, 1):
                body()
        else:
            body()

        nc.sync.dma_start(out=bh_d[:], in_=bh_t[:])

    nc.compile()
    return nc


def _get_prog():
    global _PROG
    if _PROG is None:
        _PROG = _build()
    return _PROG


def _combine(results):
    total = 0.0
    for r in results:
        total += r["bh"].astype(np.float64).sum()
    return np.float32(total / (N * S))


def kernel(predictions, targets):
    from concourse.bass_utils import run_bass_kernel_spmd

    nc = _get_prog()
    p = np.ascontiguousarray(
        np.asarray(predictions, dtype=np.float32).reshape(NCORES, NLOC * P, F))
    t = np.ascontiguousarray(
        np.asarray(targets, dtype=np.float32).reshape(NCORES, NLOC * P, F))
    in_maps = [{"p": p[k], "t": t[k]} for k in range(NCORES)]
    res = run_bass_kernel_spmd(nc, in_maps, list(range(NCORES)))
    return _combine(res.results)



# revision 6
# speedup vs baseline: 1.2947x; 1.0874x over previous
"""BerHu loss kernel for Trainium2, 8-core data-parallel.

Reference computation (per sample n over its S = 1*480*640 elements):
    d  = pred - tgt
    c  = max|d| / 5
    berhu = |d|                 where |d| <= c
          = (d^2 + c^2) / (2c)  otherwise
    loss = mean_n mean_i berhu

Identity used on device:  berhu = |d| + relu(|d| - c)^2 * (1/(2c))
Two custom DVE ops do all heavy lifting (one pass each over the data):
  ABSDIFF:  ad = |p - t|            accum: mx = max(ad)      [per partition]
  BERHU:    junk = ad + relu(ad-c)^2 * i2c   accum: bh = sum [per partition]
The host sums the per-partition/per-sample bh partials:
    loss = sum(bh) / (N * S).

Sharding: pure data parallel, 8 samples per core on 8 cores; each
sample's 307200 elements are laid out [128 partitions x 2400].
"""

import numpy as np

N = 64          # batch
S = 307200      # 1*480*640 elements per sample
NCORES = 8
NLOC = N // NCORES   # samples per core
P = 128              # SBUF partitions
F = S // P           # 2400 columns per sample

_PROG = None


def _register_ops():
    import concourse.dve_ops as dve_ops
    from concourse.dve_ops import OPS, DveOp, has_src1
    from concourse.dve_spec import (C0, C1, C2, AluOp, Spec, Src0, Src1,
                                    Zero, lower)
    from concourse.dve_spec import relu, sq, maxx
    from concourse.dve_uop import DveOpSpec

    def add_op(name, spec):
        for o in OPS:
            if o.name == name:
                return o
        op = DveOp(name, spec, subdim=False, uops_sha={})
        OPS.append(op)
        dve_ops.CUSTOM_DVE_SPECS[name] = spec
        dve_ops._SUB_OPCODE_FOR_NAME[name] = (
            dve_ops._CUSTOM_DVE_ROW_BASE + len(OPS) - 1)
        assert dve_ops._SUB_OPCODE_FOR_NAME[name] < 0x20
        for ver in ("v3", "v4"):
            sha = DveOpSpec(
                name=name,
                opcode=dve_ops.get_dve_sub_opcode(name),
                uops=lower(spec, ver=ver),
                rd1_en=has_src1(spec),
            ).sha(ver)
            op.uops_sha[ver] = sha
        return op

    def _absdiff_ref(in0, in1, c0, c1, c2):
        x = in0.astype(np.float32).reshape(in0.shape[0], -1)
        y = np.asarray(in1, np.float32).reshape(in0.shape[0], -1)
        out = np.abs(x - y).astype(np.float32)
        return out, out.max(axis=-1)

    def _berhu_ref(in0, in1, c0, c1, c2):
        # c0 = c = m/5, c1 = 1/(2c) = 2.5/m
        x = in0.astype(np.float32).reshape(in0.shape[0], -1)
        r = np.maximum(x - c0, 0.0).astype(np.float32)
        out = (x + r * r * c1).astype(np.float32)
        return out, out.sum(axis=-1, dtype=np.float32)

    d = Src0 - Src1
    absdiff = add_op(
        "ANT_BERHU_ABSDIFF",
        Spec(body=maxx(d, Zero - d), accum=AluOp.MAX, reference=_absdiff_ref),
    )
    berhu = add_op(
        "ANT_BERHU_ACC",
        Spec(body=Src0 + sq(relu(Src0 - C0)) * C1, accum=AluOp.ADD,
             reference=_berhu_ref),
    )
    return absdiff, berhu


def _build(repeat=1, loop_n=None, queues=("sync", "scalar"),
           tail_opt=False):
    """Build the per-core program. `repeat` > 1 replays the whole 8-sample
    body that many times inside one NEFF (unrolled); `loop_n` instead wraps
    the body in a device-side For_i loop (benchmarking only). `queues`:
    engine queues the paired input DMAs rotate over. `tail_opt`: load the
    last two samples as singles ([2,2,2,1,1] grouping) so only one
    sample's DVE work remains after the final DMA byte lands, shortening
    the single-shot pipeline drain by ~3 us."""
    from contextlib import ExitStack

    import concourse.bacc as bacc
    import concourse.tile as tile
    from concourse import mybir

    absdiff_op, berhu_op = _register_ops()

    f32 = mybir.dt.float32
    Alu = mybir.AluOpType

    nc = bacc.Bacc("TRN2", target_bir_lowering=False, debug=False,
                   num_devices=NCORES)
    p_d = nc.dram_tensor("p", [NLOC * P, F], f32, kind="ExternalInput").ap()
    t_d = nc.dram_tensor("t", [NLOC * P, F], f32, kind="ExternalInput").ap()
    bh_d = nc.dram_tensor("bh", [P, NLOC], f32, kind="ExternalOutput").ap()

    with tile.TileContext(nc) as tc, ExitStack() as ctx:
        io = ctx.enter_context(tc.tile_pool(name="io", bufs=3))
        work = ctx.enter_context(tc.tile_pool(name="work", bufs=3))
        work2 = ctx.enter_context(tc.tile_pool(name="work2", bufs=2))
        small = ctx.enter_context(tc.tile_pool(name="small", bufs=3))
        stats = ctx.enter_context(tc.tile_pool(name="stats", bufs=1))
        psum = ctx.enter_context(tc.tile_pool(name="psum", bufs=3,
                                              space="PSUM"))

        bh_t = stats.tile([P, NLOC], f32, tag="bh")
        ones_t = stats.tile([1, P], f32, tag="ones")
        nc.vector.memset(ones_t[:], 1.0)
        # identity matrix for PE cross-partition transpose
        ident = stats.tile([P, P], f32, tag="ident")
        nc.vector.memset(ident[:], 1.0)
        nc.gpsimd.affine_select(
            out=ident[:], in_=ident[:], pattern=[[-1, P]],
            compare_op=mybir.AluOpType.is_equal, fill=0.0,
            base=0, channel_multiplier=1,
        )
        total = NLOC * repeat
        q_engines = [getattr(nc, q) for q in queues]

        pair = {}

        def load_pair(i):
            # one 2.4 MB DMA covers two consecutive samples (halves the
            # per-sample DMA instruction + completion-sem overhead)
            n = i % NLOC
            rows = slice(n * P, (n + 2) * P)
            pt = io.tile([P, 2 * F], f32, tag="p")
            tt = io.tile([P, 2 * F], f32, tag="t")
            src_p = p_d[rows, :].rearrange("(s p) f -> p s f", s=2)
            src_t = t_d[rows, :].rearrange("(s p) f -> p s f", s=2)
            q_engines[i % len(q_engines)].dma_start(
                out=pt[:].rearrange("p (s f) -> p s f", s=2), in_=src_p)
            q_engines[(i + 1) % len(q_engines)].dma_start(
                out=tt[:].rearrange("p (s f) -> p s f", s=2), in_=src_t)
            pair["p"], pair["t"] = pt, tt

        singles = {}

        def load_single(n):
            # dedicated [P, F] tiles for a tail sample (1.2 MB DMAs)
            rows = slice((n % NLOC) * P, (n % NLOC + 1) * P)
            pt = stats.tile([P, F], f32, tag=f"s{n % NLOC}p")
            tt = stats.tile([P, F], f32, tag=f"s{n % NLOC}t")
            q_engines[0].dma_start(out=pt[:], in_=p_d[rows, :])
            q_engines[1 % len(q_engines)].dma_start(
                out=tt[:], in_=t_d[rows, :])
            singles[n % NLOC] = {"p": pt, "t": tt}

        def pass1(i, single=False):
            if single:
                src_p = singles[i % NLOC]["p"][:, :]
                src_t = singles[i % NLOC]["t"][:, :]
            else:
                if i % 2 == 0:
                    load_pair(i)
                k = i % 2
                cols = slice(k * F, (k + 1) * F)
                src_p = pair["p"][:, cols]
                src_t = pair["t"][:, cols]
            # ad = |p - t|; mxn = per-partition max(ad)
            ad = work.tile([P, F], f32, tag="ad")
            mxn = small.tile([P, 1], f32, tag="mxn")
            nc.vector._custom_dve(absdiff_op, out=ad[:],
                                  in0=src_p, in1=src_t,
                                  accum_out=mxn[:])
            return {"ad": ad, "mxn": mxn}

        def chain(st):
            # cross-partition max: PE transpose (mxn^T @ I) -> [1, P] PSUM
            # row, DVE max-reduce -> scalar m; cpair = (m/5, 2.5/m); a K=1
            # ones-matmul broadcasts cpair to all 128 partitions in PSUM.
            mrow = psum.tile([1, P], f32, tag="mrow")
            nc.tensor.matmul(mrow[:], st["mxn"][:], ident[:],
                             start=True, stop=True)
            cpair = small.tile([1, 2], f32, tag="cpair")
            mr = small.tile([1, 3], f32, tag="mr")
            nc.vector.tensor_reduce(out=mr[0:1, 0:1], in_=mrow[:],
                                    axis=mybir.AxisListType.X, op=Alu.max)
            # floor m to avoid 1/0 when pred == tgt exactly (then bh = 0
            # correctly, since relu(0 - c) = 0)
            nc.vector.tensor_scalar_max(out=mr[0:1, 1:2],
                                        in0=mr[0:1, 0:1], scalar1=1e-20)
            nc.vector.reciprocal(out=mr[0:1, 2:3], in_=mr[0:1, 1:2])
            nc.vector.tensor_scalar_mul(out=cpair[0:1, 0:1],
                                        in0=mr[0:1, 1:2], scalar1=0.2)
            nc.vector.tensor_scalar_mul(out=cpair[0:1, 1:2],
                                        in0=mr[0:1, 2:3], scalar1=2.5)
            cb = psum.tile([P, 2], f32, tag="cb")
            nc.tensor.matmul(cb[:], ones_t[:], cpair[:],
                             start=True, stop=True)
            st["cb"] = cb

        def pass2(i, st):
            # bh[:, n] = sum(ad + relu(ad - c)^2 * i2c)
            n = i % NLOC
            junk = work2.tile([P, F], f32, tag="junk")
            nc.vector._custom_dve(berhu_op, out=junk[:], in0=st["ad"][:],
                                  s0=st["cb"][:, 0:1], s1=st["cb"][:, 1:2],
                                  accum_out=bh_t[:, n:n + 1])

        # 2-deep software pipeline: pass1(i) | chain(i-1) | pass2(i-2) keeps
        # the DVE stream free of waits on the c-derivation chain.
        def body():
            if not tail_opt:
                hist = {}
                for i in range(total):
                    hist[i] = pass1(i)
                    if i - 1 >= 0:
                        chain(hist[i - 1])
                    if i - 2 >= 0:
                        pass2(i - 2, hist.pop(i - 2))
                for i in (total - 2, total - 1):
                    if i >= 0:
                        if "cb" not in hist[i]:
                            chain(hist[i])
                        pass2(i, hist.pop(i))
                return
            # tail-lean: samples 0-5 as pairs, 6 and 7 as singles whose
            # DMAs trail the stream; the DVE finishes samples 0-6 before
            # the last byte lands, leaving only sample 7's work as drain.
            assert total % NLOC == 0
            for b in range(0, total, NLOC):
                hist = {}
                for j in range(6):
                    hist[j] = pass1(b + j)
                    if j == 4:
                        load_single(b + 6)
                        load_single(b + 7)
                    if j - 1 >= 0:
                        chain(hist[j - 1])
                    if j - 2 >= 0:
                        pass2(b + j - 2, hist.pop(j - 2))
                chain(hist[5])
                pass2(b + 4, hist.pop(4))
                pass2(b + 5, hist.pop(5))
                for j in (6, 7):
                    st = pass1(b + j, single=True)
                    chain(st)
                    pass2(b + j, st)

        if loop_n is not None:
            with tc.For_i(0, loop_n, 1):
                body()
        else:
            body()

        nc.sync.dma_start(out=bh_d[:], in_=bh_t[:])

    nc.compile()
    return nc


def _get_prog():
    global _PROG
    if _PROG is None:
        _PROG = _build()
    return _PROG


def _combine(results):
    total = 0.0
    for r in results:
        total += r["bh"].astype(np.float64).sum()
    return np.float32(total / (N * S))


def kernel(predictions, targets):
    from concourse.bass_utils import run_bass_kernel_spmd

    nc = _get_prog()
    p = np.ascontiguousarray(
        np.asarray(predictions, dtype=np.float32).reshape(NCORES, NLOC * P, F))
    t = np.ascontiguousarray(
        np.asarray(targets, dtype=np.float32).reshape(NCORES, NLOC * P, F))
    in_maps = [{"p": p[k], "t": t[k]} for k in range(NCORES)]
    res = run_bass_kernel_spmd(nc, in_maps, list(range(NCORES)))
    return _combine(res.results)

